# revision 15
# baseline (speedup 1.0000x reference)
"""LoraLinear (int8-dequant matmul + low-rank LoRA) on 8 trn2 NeuronCores.

out[b,s,o] = sum_i x[b,s,i]*q[o,i]*scale[o] + 2.0 * sum_r (sum_i x[b,s,i]*A[r,i]) * B[o,r]

Strategy: data-parallel over the 8192 flattened tokens (1024/core, no
collectives). Host folds scale into the weight and splits both x and w
into fp8e4m3 (hi + residual) pairs: w ~ w1 + w2, x ~ x1 + x2. The device
computes x1@w1 + x2@w1 + x1@w2 with DoubleRow fp8 matmuls (2 k-chunks of
128 per instruction at 0.5 cycles/row — 4x the bf16 MAC rate). The x1@w2
correction runs on only 12 of 16 chunk pairs: the dropped pairs raise
rel err to ~1.33e-2 (vs the 2e-2 gate) and cut the main-matmul cost to
2.75/4 of bf16 streaming. The LoRA path is fp8 DoubleRow too (A and 2B^T
quantized to fp8, xa re-quantized on eviction), folded into the same
PSUM accumulation group as one extra K=64 matmul per output tile.

Schedule: phase 1 (ot=0) interleaves xa (first token half) with 7 of 8
token groups so the PE keeps pace with the x/w DMA stream; the second xa
half reuses the freed PSUM bank right after, when all of x1 is resident.
b2 arrives per-ot so ot=0 never waits on the full LoRA-B load; each ot
prefetches the next ot's weights 3 token-tiles early; evictions split
into two staging tiles so DVE and ACT run the two halves concurrently.
The w2-dropped pairs sit at icp {6,7,14,15}, matching the lighter DMA
supply mid-stream and at the tail of the phase-1 interleave.
"""

import numpy as np
import ml_dtypes

BF16 = ml_dtypes.bfloat16
F8 = ml_dtypes.float8_e4m3

B, S, DIN, DOUT, R = 4, 2048, 4096, 4096, 64
N_CORES = 8
TOK = B * S  # 8192
T = TOK // N_CORES  # 1024 tokens per core
P = 128
IC = DIN // P  # 32 contraction chunks of 128
ICP = IC // 2  # 16 chunk pairs (DoubleRow does 2 chunks/instr)
# Per-ot w2-drop sets: 40 dropped (pair, ot) cells total, none at ot=0
# (phase 1 is DMA-bound there, so corrections are free), six each at
# ots 1-5 and five at ots 6-7 where the PE is the binding resource.
D6 = (2, 3, 6, 7, 14, 15)
D5 = (3, 6, 7, 14, 15)
W2_DROPS = {0: (), 1: D6, 2: D6, 3: D6, 4: D6, 5: D6, 6: D5, 7: D5}
O_TILE = 512
N_OT = DOUT // O_TILE  # 8
N_TT = T // P  # 8
SCALING = 2.0

_CACHE = {}


def build_nc():
    import concourse.mybir as mybir
    import concourse.tile as tile
    from concourse import bacc

    dt = mybir.dt
    DR = mybir.MatmulPerfMode.DoubleRow
    nc = bacc.Bacc("TRN2", target_bir_lowering=False, debug=False,
                   num_devices=N_CORES)

    x1_d = nc.dram_tensor("x1", [P, IC, T], dt.float8e4, kind="ExternalInput").ap()
    x2_d = nc.dram_tensor("x2", [P, IC, T], dt.float8e4, kind="ExternalInput").ap()
    w1_d = nc.dram_tensor("w1", [N_OT, P, IC, O_TILE], dt.float8e4, kind="ExternalInput").ap()
    w2_d = nc.dram_tensor("w2", [N_OT, P, IC, O_TILE], dt.float8e4, kind="ExternalInput").ap()
    aT_d = nc.dram_tensor("aT", [P, IC, R], dt.float8e4, kind="ExternalInput").ap()
    b2T_d = nc.dram_tensor("b2T", [R // 2, 2, DOUT], dt.float8e4, kind="ExternalInput").ap()
    out_d = nc.dram_tensor("out", [N_OT, N_TT, P, O_TILE], dt.bfloat16, kind="ExternalOutput").ap()

    XCH = 2   # ic per x tile chunk -> 16 chunks per part (one DoubleRow pair)
    WCH = 4   # ic per w tile chunk -> 8 chunks (w1) / 6 chunks (w2)
    NW1 = IC // WCH

    def w2q(ot):
        return sorted({(2 * p) // WCH for p in range(ICP) if p not in W2_DROPS[ot]})

    with tile.TileContext(nc) as tc:
        with (
            tc.tile_pool(name="xpool", bufs=1) as xpool,
            tc.tile_pool(name="wpool", bufs=2) as wpool,
            tc.tile_pool(name="cpool", bufs=1) as cpool,
            tc.tile_pool(name="opool", bufs=6) as opool,
            tc.tile_pool(name="psmain", bufs=7, space="PSUM") as psmain,
            tc.tile_pool(name="psxa", bufs=1, space="PSUM") as psxa,
        ):
            # x and a split into independently-DMA'd tiles so PE can stream
            # behind the loads (Tile deps are tile-granular).
            ACH = 8
            ats = [cpool.tile([P, ACH, R], dt.float8e4, tag=f"at{i}", name=f"at{i}")
                   for i in range(IC // ACH)]
            x1ts = [xpool.tile([P, XCH, T], dt.float8e4, tag=f"x1t{i}", name=f"x1t{i}")
                    for i in range(ICP)]
            x2ts = [xpool.tile([P, XCH, T], dt.float8e4, tag=f"x2t{i}", name=f"x2t{i}")
                    for i in range(ICP)]
            b2ts = [cpool.tile([R // 2, 2, O_TILE], dt.float8e4, tag=f"b2_{o}", name=f"b2_{o}")
                    for o in range(N_OT)]

            def x1_sl(icp, lo, hi):
                return x1ts[icp][:, :, lo:hi]

            def x2_sl(icp, lo, hi):
                return x2ts[icp][:, :, lo:hi]

            def a_sl(icp):
                # DoubleRow pair of A chunks: [P, 2, R]
                ic = 2 * icp
                return ats[ic // ACH][:, ic % ACH:ic % ACH + 2, :]

            def w_tiles(ot):
                ws1 = [wpool.tile([P, WCH, O_TILE], dt.float8e4, tag=f"w1{q}", name=f"w1_{q}")
                       for q in range(NW1)]
                ws2 = {q: wpool.tile([P, WCH, O_TILE], dt.float8e4, tag=f"w2{q}", name=f"w2_{q}")
                       for q in w2q(ot)}
                for q in range(NW1):
                    nc.sync.dma_start(ws1[q][:], w1_d[ot, :, WCH * q:WCH * (q + 1), :])
                    if q in ws2:
                        nc.sync.dma_start(ws2[q][:], w2_d[ot, :, WCH * q:WCH * (q + 1), :])
                return ws1, ws2

            def w_sl(ws, icp):
                ic = 2 * icp
                if isinstance(ws, dict):
                    return ws[ic // WCH][:, ic % WCH:ic % WCH + 2, :]
                return ws[ic // WCH][:, ic % WCH:ic % WCH + 2, :]

            # ACT warmup: a dummy 1-row copy forces the activation-table load
            # (1.3us) to happen now, while ACT is idle, instead of on the
            # critical xa-eviction path mid-kernel.
            warm = cpool.tile([1, 8], dt.float32, tag="warm", name="warm")
            warm2 = cpool.tile([1, 8], dt.float32, tag="warm2", name="warm2")
            nc.any.memset(warm[:], 0.0)
            nc.scalar.copy(warm2[:], warm[:])

            # PE warmup: dummy DoubleRow matmuls on memset tiles keep the PE
            # busy through the initial DMA latency so its p-state clock is
            # fully ramped (3us of continuous use) when real data lands.
            wsrc = cpool.tile([P, 2, P], dt.float8e4, tag="wsrc", name="wsrc")
            nc.vector.memset(wsrc[:], 0.0)
            pswarm = psxa.tile([R, O_TILE], dt.float32, tag="psxa", name="pswarm")
            for i in range(55):
                nc.tensor.matmul(pswarm[0:P // 2, 0:P // 2], wsrc[:, :, 0:P // 2],
                                 wsrc[:, :, 0:P // 2], start=(i == 0),
                                 stop=(i == 54), perf_mode=DR)

            # phase-0 DMA emission, hand-ordered to the phase-1 consumption
            # pattern: x pair j feeds icp j; w1 chunk q is needed at icp 2q,
            # w2 chunk q at its first non-dropped icp; a chunk k at icp 4k.
            w01 = [wpool.tile([P, WCH, O_TILE], dt.float8e4, tag=f"w1{q}", name=f"w01_{q}")
                   for q in range(NW1)]
            w02 = {q: wpool.tile([P, WCH, O_TILE], dt.float8e4, tag=f"w2{q}", name=f"w02_{q}")
                   for q in w2q(0)}
            # first x chunk pair split into token-half DMAs (subtile deps)
            # so the very first xa/main matmuls wait on a 512-token transfer
            H = T // 2
            nc.sync.dma_start(ats[0][:], aT_d[:, 0:ACH, :])
            nc.sync.dma_start(x1ts[0][:, :, 0:H], x1_d[:, 0:XCH, 0:H])
            nc.sync.dma_start(w01[0][:], w1_d[0, :, 0:WCH, :])
            nc.sync.dma_start(w02[0][:], w2_d[0, :, 0:WCH, :])
            nc.sync.dma_start(x2ts[0][:, :, 0:H], x2_d[:, 0:XCH, 0:H])
            nc.sync.dma_start(x1ts[0][:, :, H:T], x1_d[:, 0:XCH, H:T])
            nc.sync.dma_start(x2ts[0][:, :, H:T], x2_d[:, 0:XCH, H:T])
            nc.sync.dma_start(b2ts[0][:], b2T_d[:, :, 0:O_TILE])
            w1_q = list(range(1, NW1))
            w2_q = [q for q in w2q(0) if q != 0]
            for j in range(1, ICP):
                nc.sync.dma_start(x1ts[j][:], x1_d[:, XCH * j:XCH * (j + 1), :])
                nc.sync.dma_start(x2ts[j][:], x2_d[:, XCH * j:XCH * (j + 1), :])
                if j % 2 == 1 and w1_q:
                    q = w1_q.pop(0)
                    nc.sync.dma_start(w01[q][:], w1_d[0, :, WCH * q:WCH * (q + 1), :])
                elif j % 2 == 0 and w2_q:
                    q = w2_q.pop(0)
                    nc.sync.dma_start(w02[q][:], w2_d[0, :, WCH * q:WCH * (q + 1), :])
                if j == 3:
                    nc.sync.dma_start(ats[1][:], aT_d[:, ACH:2 * ACH, :])
                elif j == 6:
                    nc.sync.dma_start(ats[2][:], aT_d[:, 2 * ACH:3 * ACH, :])
                elif j == 9:
                    nc.sync.dma_start(ats[3][:], aT_d[:, 3 * ACH:4 * ACH, :])
            for o in range(1, N_OT):
                nc.sync.dma_start(b2ts[o][:], b2T_d[:, :, o * O_TILE:(o + 1) * O_TILE])

            # xa stored fp8 as two token-half tiles [32, 2, 512]: row
            # r = h*32 + p (DoubleRow slots). Separate tiles let the first
            # LoRA matmuls start before the second half is evicted.
            xaT0 = cpool.tile([R // 2, 2, O_TILE], dt.float8e4, tag="xaT0", name="xaT0")
            xaT1 = cpool.tile([R // 2, 2, O_TILE], dt.float8e4, tag="xaT1", name="xaT1")
            Q = R // 2

            def xa_sl(tt):
                if tt < N_TT // 2:
                    return xaT0[:, :, tt * P:(tt + 1) * P]
                return xaT1[:, :, tt * P - O_TILE:(tt + 1) * P - O_TILE]

            def lora_and_evict(ps, ot, tt, final=False):
                nc.tensor.matmul(
                    ps[:], xa_sl(tt), b2ts[ot][:],
                    start=False, stop=True, perf_mode=DR,
                )
                if final:
                    # halves evicted by DVE/ACT and stored via BOTH hwdge
                    # queues (SP + Activation) so the two end-of-kernel store
                    # descriptors process in parallel
                    h2 = O_TILE // 2
                    stf1 = opool.tile([P, h2], dt.bfloat16, tag="st", name="stf1")
                    stf2 = opool.tile([P, h2], dt.bfloat16, tag="st", name="stf2")
                    nc.vector.tensor_copy(out=stf1[:], in_=ps[:, :h2])
                    nc.sync.dma_start(out_d[ot, tt, :, 0:h2], stf1[:])
                    nc.scalar.copy(stf2[:], ps[:, h2:])
                    nc.scalar.dma_start(out_d[ot, tt, :, h2:O_TILE], stf2[:])
                    return
                # two staging tiles so DVE and ACT evict halves concurrently
                h = O_TILE // 2
                st1 = opool.tile([P, h], dt.bfloat16, tag="st", name="st1")
                st2 = opool.tile([P, h], dt.bfloat16, tag="st", name="st2")
                nc.vector.tensor_copy(out=st1[:], in_=ps[:, :h])
                nc.sync.dma_start(out_d[ot, tt, :, 0:h], st1[:])
                nc.scalar.copy(st2[:], ps[:, h:])
                nc.sync.dma_start(out_d[ot, tt, :, h:O_TILE], st2[:])

            def main_mms(ps, icp, x_lo, x_hi, ws1, ws2, start, drop):
                # x1@w1 + x2@w1 (+ x1@w2 on non-dropped pairs)
                nc.tensor.matmul(ps[:], x1_sl(icp, x_lo, x_hi), w_sl(ws1, icp),
                                 start=start, stop=False, perf_mode=DR)
                nc.tensor.matmul(ps[:], x2_sl(icp, x_lo, x_hi), w_sl(ws1, icp),
                                 start=False, stop=False, perf_mode=DR)
                if icp not in drop:
                    nc.tensor.matmul(ps[:], x1_sl(icp, x_lo, x_hi), w_sl(ws2, icp),
                                     start=False, stop=False, perf_mode=DR)

            def xa_mms(ps_ap, tb):
                for icp in range(ICP):
                    nc.tensor.matmul(
                        ps_ap, a_sl(icp),
                        x1_sl(icp, tb * O_TILE, (tb + 1) * O_TILE),
                        start=(icp == 0), stop=(icp == ICP - 1), perf_mode=DR,
                    )

            def xa_evict(ps, tb):
                xt = xaT0 if tb == 0 else xaT1
                nc.vector.tensor_copy(out=xt[:, 0, :], in_=ps[0:Q, :])
                nc.scalar.copy(xt[:, 1, :], ps[Q:R, :])

            # ---- phase 1 (ot=0): icp-outer, xa (token half 0) + 7 token
            # groups interleaved so the PE tracks the x/w DMA stream
            NPG = 7
            ps_g = [psmain.tile([P, O_TILE], dt.float32, tag="ps", name=f"psg{g}") for g in range(NPG)]
            ps_xa = psxa.tile([R, O_TILE], dt.float32, tag="psxa", name="psxa0")
            for icp in range(ICP):
                nc.tensor.matmul(
                    ps_xa[:], a_sl(icp), x1_sl(icp, 0, O_TILE),
                    start=(icp == 0), stop=(icp == ICP - 1), perf_mode=DR,
                )
                for tt in range(NPG):
                    main_mms(ps_g[tt], icp, tt * P, (tt + 1) * P, w01, w02,
                             start=(icp == 0), drop=W2_DROPS[0])
            xa_evict(ps_xa, 0)
            # prefetch ot=1 weights now: their DMAs queue behind the phase-1
            # stream and load while the PE finishes ot=0
            pending = w_tiles(1)
            # first-half LoRAs only need xaT0; they also free psmain banks
            for tt in range(4):
                lora_and_evict(ps_g[tt], 0, tt)
            # second xa token half lands in a freed main bank; x1 is resident
            ps_xa2 = psmain.tile([P, O_TILE], dt.float32, tag="ps", name="psxa1")
            xa_mms(ps_xa2[0:R, :], 1)
            xa_evict(ps_xa2, 1)
            # ot=0 last token group: runs while xaT1 is being evicted
            ps7 = psmain.tile([P, O_TILE], dt.float32, tag="ps", name="ps7")
            for icp in range(ICP):
                main_mms(ps7, icp, (N_TT - 1) * P, N_TT * P, w01, w02,
                         start=(icp == 0), drop=W2_DROPS[0])
            for tt in range(4, NPG):
                lora_and_evict(ps_g[tt], 0, tt)
            lora_and_evict(ps7, 0, N_TT - 1)

            # ---- steady state: ot = 1..7, next-ot weights prefetched early
            for ot in range(1, N_OT):
                ws1, ws2 = pending
                for tt in range(N_TT):
                    if tt == N_TT - 3 and ot < N_OT - 1:
                        pending = w_tiles(ot + 1)
                    ps = psmain.tile([P, O_TILE], dt.float32, tag="ps", name="ps")
                    for icp in range(ICP):
                        main_mms(ps, icp, tt * P, (tt + 1) * P, ws1, ws2,
                                 start=(icp == 0), drop=W2_DROPS[ot])
                    lora_and_evict(ps, ot, tt,
                                   final=(ot == N_OT - 1 and tt == N_TT - 1))

    nc.compile()
    return nc


def _split_f8(a):
    """Split float32 array into fp8e4m3 hi + residual (a ~ hi + lo)."""
    hi = a.astype(F8)
    lo = (a - hi.astype(np.float32)).astype(F8)
    return hi, lo


def _prep_inputs(x, qweight, scale, lora_A, lora_B):
    x_flat = np.ascontiguousarray(x.reshape(TOK, DIN))
    # x per core: [P, IC, T], row i = ic*P + p
    xT_all = x_flat.T.astype(np.float32)  # [DIN, TOK]
    per_core_x1, per_core_x2 = [], []
    for c in range(N_CORES):
        xs = xT_all[:, c * T:(c + 1) * T]
        h, l = _split_f8(xs)
        per_core_x1.append(np.ascontiguousarray(
            h.reshape(IC, P, T).transpose(1, 0, 2)))
        per_core_x2.append(np.ascontiguousarray(
            l.reshape(IC, P, T).transpose(1, 0, 2)))
    # weight with scale folded, transposed: wT[i, o]; fp8 hi/lo split
    w = qweight.astype(np.float32) * scale.astype(np.float32)  # [DOUT, DIN]
    wT = np.ascontiguousarray(w.T)  # [DIN, DOUT]
    w1, w2 = _split_f8(wT)
    w1_t = np.ascontiguousarray(
        w1.reshape(IC, P, N_OT, O_TILE).transpose(2, 1, 0, 3))  # [N_OT, P, IC, O_TILE]
    w2_t = np.ascontiguousarray(
        w2.reshape(IC, P, N_OT, O_TILE).transpose(2, 1, 0, 3))
    aT = np.ascontiguousarray(
        lora_A.T.astype(F8).reshape(IC, P, R).transpose(1, 0, 2))  # [P, IC, R]
    # 2*B^T as [32, 2, DOUT] fp8: row r = h*32 + p
    b2 = (SCALING * lora_B).T.astype(F8)  # [R, DOUT]
    b2T = np.ascontiguousarray(b2.reshape(2, R // 2, DOUT).transpose(1, 0, 2))
    return per_core_x1, per_core_x2, w1_t, w2_t, aT, b2T


def run(x, qweight, scale, lora_A, lora_B, trace=False):
    from concourse.bass_utils import run_bass_kernel_spmd

    if "nc" not in _CACHE:
        _CACHE["nc"] = build_nc()
    nc = _CACHE["nc"]

    x1s, x2s, w1_t, w2_t, aT, b2T = _prep_inputs(x, qweight, scale, lora_A, lora_B)
    in_maps = [
        {"x1": x1s[c], "x2": x2s[c], "w1": w1_t, "w2": w2_t, "aT": aT, "b2T": b2T}
        for c in range(N_CORES)
    ]
    res = run_bass_kernel_spmd(nc, in_maps, core_ids=list(range(N_CORES)),
                               trace=trace)
    outs = []
    for c in range(N_CORES):
        o = res.results[c]["out"]  # [N_OT, N_TT, P, O_TILE]
        outs.append(o.transpose(1, 2, 0, 3).reshape(T, DOUT))
    full = np.concatenate(outs, axis=0).reshape(B, S, DOUT).astype(np.float32)
    return full, res


def kernel(x, qweight, scale, lora_A, lora_B):
    full, _ = run(x, qweight, scale, lora_A, lora_B)
    return full
